# revision 32
# baseline (speedup 1.0000x reference)
"""Trainium2 Bass kernel for the supervised-contrastive loss (nn_KCL_69784628626020).

Strategy (8 NeuronCores, SPMD, zero collectives):
  - Shard anchors (rows of q, k, y) across cores: 1024 rows/core.
  - Class counts / weights are computed on the HOST (pure input marshalling):
    no on-device histogram, no AllReduce, no cross-core sync at all.
  - Each core computes its [1024, 8192] slab of S = q_loc @ q_full^T with
    fp8e4 DoubleRow matmuls (q pre-scaled by 16; exp scale absorbs the 256x).
  - Column tiles are ROTATED per core (tile t covers original column tile
    (r+t) mod 8), so the diagonal block always lands in tile t=0 and the
    diagonal-kill matmul (-240*16 at the diagonal entry, pushing exp to 0)
    is issued for t=0 only.
  - The per-column weight w_j = 1/count(y_j) is folded INTO the contraction:
    q's last two feature dims are dropped (zero-mean noise comparable to the
    fp8 quantization) and replaced by ones on the stationary side and
    X1,X2 (fp8 value + residual of 256*tau*ln(w_j)) on the moving side, so
    S picks up +tau*ln(w_j) with ZERO extra matmuls. ScalarE then computes
    EW = w_j * exp(S/tau) directly and its accum_out produces
    AW_i = sum_j w_j E_ij for free.
  - Per row i and tile the only DVE pass is
        BU'_i += sum_{y_j==y_i} EW_ij    (STT is_equal+mult on DVE)
    with BU'_i = w_i * BU_i (w constant within a class).
  - den_i = log(AW_i - BU'_i); num_i = log(kpos_i + c_i*BU'_i)
    loss_i = (den_i - num_i) / (c_i - 1 + K)
  - kpos via fp8 DoubleRow matmuls + exp + masked STT row-reduce.
  - Final mean: ones-matmul partition reduction -> per-core partial; host
    adds the 8 partials (the unshard step).
"""

import numpy as np
from contextlib import ExitStack

import concourse.bass as bass
import concourse.bacc as bacc
import concourse.tile as tile
from concourse import mybir
from concourse.bass_utils import run_bass_kernel_spmd
import ml_dtypes

F32 = mybir.dt.float32
F16 = mybir.dt.float16
BF16 = mybir.dt.bfloat16
FP8 = mybir.dt.float8e4

TAU = 0.07
NCORES = 8
QSCALE = 16.0          # q,k pre-scale before fp8 quantization
DIAG_Z = -240.0        # zsel value (TRN fp8e4 max magnitude)
DIAG_W = 16.0          # wdg value; product -3840 kills the diagonal exp
NUM_CLASSES = 1000


class Cfg:
    def __init__(self, N=8192, D=512, KP=8, TW=2048, ncores=NCORES):
        self.N = N            # total rows (anchors)
        self.D = D            # feature dim
        self.KP = KP          # external positives per anchor
        self.TW = TW          # column tile width (2 PSUM banks wide x f32)
        self.ncores = ncores
        self.NL = N // ncores     # rows per core
        self.NB = self.NL // 128  # row blocks per core
        self.NS = N // TW         # column tiles
        self.KC = D // 128        # 128-contraction chunks
        assert self.NL % 128 == 0 and N % TW == 0 and D % 128 == 0
        assert TW % 512 == 0 and self.KC % 2 == 0
        assert TW % self.NL == 0  # diagonal lands in the head of tile t=0
        self.NCH = TW // 512      # 512-wide psum chunks per column tile
        assert KP * 128 <= 1024


def build_bass(cfg: Cfg, e_bufs=10):
    N, D, KP, TW = cfg.N, cfg.D, cfg.KP, cfg.TW
    NL, NB, NS, KC, NCH = cfg.NL, cfg.NB, cfg.NS, cfg.KC, cfg.NCH

    nc = bacc.Bacc("TRN2", target_bir_lowering=False, debug=False,
                   num_devices=cfg.ncores)

    KW = KP * 128  # k-path tile width per row block
    KWCH = KW // 512

    # ---- kernel I/O -------------------------------------------------------
    qTr_d = nc.dram_tensor("qTr", [KC, 128, N], FP8, kind="ExternalInput")
    qTl_d = nc.dram_tensor("qTl", [KC, 128, NL], FP8, kind="ExternalInput")
    kT_d = nc.dram_tensor("kT", [NB, KC, 128, KW], FP8, kind="ExternalInput")
    ybc_d = nc.dram_tensor("ybc", [128, N], F16, kind="ExternalInput")
    yrow_d = nc.dram_tensor("yrow", [128, NB], F32, kind="ExternalInput")
    wdg_d = nc.dram_tensor("wdg", [128, NL + (NB - 1) * 128], FP8,
                           kind="ExternalInput")
    zsel_d = nc.dram_tensor("zsel", [128, 128], FP8, kind="ExternalInput")
    mask8_d = nc.dram_tensor("mask8", [128, KW], F16, kind="ExternalInput")
    cloc_d = nc.dram_tensor("cloc", [128, NB], F32, kind="ExternalInput")
    dinv_d = nc.dram_tensor("dinv", [128, NB], F32, kind="ExternalInput")
    out_d = nc.dram_tensor("out", [1, 1], F32, kind="ExternalOutput")

    ESC = float(1.0 / (QSCALE * QSCALE * TAU))  # exp scale

    with tile.TileContext(nc) as tc, ExitStack() as ctx:
        const = ctx.enter_context(tc.tile_pool(name="const", bufs=1))
        rh_pool = ctx.enter_context(tc.tile_pool(name="rh", bufs=2))
        psum_pool = ctx.enter_context(tc.tile_pool(name="ps", bufs=2, space="PSUM"))
        ew_pool = ctx.enter_context(tc.tile_pool(name="ew", bufs=e_bufs))
        busc_pool = ctx.enter_context(tc.tile_pool(name="busc", bufs=2))
        kt_pool = ctx.enter_context(tc.tile_pool(name="kt", bufs=2))
        ek_pool = ctx.enter_context(tc.tile_pool(name="ek", bufs=2))

        # ---- resident constants (k-path operands first so PE starts) -----
        qtl = const.tile([128, KC, NL], FP8, tag="qtl")
        for c in range(KC):
            nc.sync.dma_start(qtl[:, c, :], qTl_d[c, :, :])
        mask8 = const.tile([128, KW], F16, tag="mask8")
        nc.sync.dma_start(mask8[:, :], mask8_d[:, :])
        yrow = const.tile([128, NB], F32, tag="yrow")
        nc.sync.dma_start(yrow[:, :], yrow_d[:, :])
        wdg = const.tile([128, NL + (NB - 1) * 128], FP8, tag="wdg")
        nc.sync.dma_start(wdg[:, :], wdg_d[:, :])
        zsel = const.tile([128, 128], FP8, tag="zsel")
        nc.sync.dma_start(zsel[:, :], zsel_d[:, :])
        ybc = const.tile([128, N], F16, tag="ybc")
        cloc = const.tile([128, NB], F32, tag="cloc")
        dinv = const.tile([128, NB], F32, tag="dinv")
        ones_col = const.tile([128, 1], F32, tag="ones_col")
        nc.vector.memset(ones_col[:, :], 1.0)

        # accumulator slots
        awslt = const.tile([128, NB * NS], F32, tag="awslt")
        buslt = const.tile([128, NB * NS], F32, tag="buslt")
        kpos = const.tile([128, NB], F32, tag="kpos")
        losscol = const.tile([128, NB], F32, tag="losscol")

        # ---- k-path: kpos_i = sum_k exp(q.k/TAU) -------------------------
        for b in range(NB):
            kt = kt_pool.tile([128, KC, KW], FP8, tag="kt")
            for c in range(KC):
                nc.sync.dma_start(kt[:, c, :], kT_d[b, c, :, :])
            kps = psum_pool.tile([128, TW], F32, name="kps", tag="ps_t")
            for dc in range(KC // 2):
                for nch in range(KWCH):
                    nc.tensor.matmul(
                        kps[:, nch * 512:(nch + 1) * 512],
                        qtl[:, 2 * dc:2 * dc + 2, b * 128:(b + 1) * 128],
                        kt[:, 2 * dc:2 * dc + 2, nch * 512:(nch + 1) * 512],
                        start=(dc == 0), stop=(dc == KC // 2 - 1),
                        perf_mode=mybir.MatmulPerfMode.DoubleRow)
            ek = ek_pool.tile([128, KW], BF16, tag="ek")
            nc.scalar.activation(ek[:, :], kps[:, 0:KW],
                                 mybir.ActivationFunctionType.Exp, scale=ESC)
            nc.vector.scalar_tensor_tensor(
                ek[:, :], mask8[:, :], 1.0, ek[:, :],
                op0=mybir.AluOpType.mult, op1=mybir.AluOpType.mult,
                accum_out=kpos[:, b:b + 1])

        # deferred big/late constants (needed from the first BU STT on)
        nc.sync.dma_start(ybc[:, :], ybc_d[:, :])
        nc.sync.dma_start(cloc[:, :], cloc_d[:, :])
        nc.sync.dma_start(dinv[:, :], dinv_d[:, :])

        # ---- main loop: score slab (rotated column tiles) ----------------
        for t in range(NS):
            rhs = rh_pool.tile([128, KC, TW], FP8, tag="rh", name=f"rhs{t}")
            for c in range(KC):
                nc.sync.dma_start(rhs[:, c, :], qTr_d[c, :, t * TW:(t + 1) * TW])
            for b in range(NB):
                nch_b = (b * 128) // 512  # psum chunk holding the diagonal
                ps = psum_pool.tile([128, TW], F32, name="ps", tag="ps_t")
                for nch in range(NCH):
                    diag_here = (t == 0 and nch == nch_b)
                    for dc in range(KC // 2):
                        last = (dc == KC // 2 - 1)
                        nc.tensor.matmul(
                            ps[:, nch * 512:(nch + 1) * 512],
                            qtl[:, 2 * dc:2 * dc + 2, b * 128:(b + 1) * 128],
                            rhs[:, 2 * dc:2 * dc + 2, nch * 512:(nch + 1) * 512],
                            start=(dc == 0), stop=(last and not diag_here),
                            perf_mode=mybir.MatmulPerfMode.DoubleRow)
                    if diag_here:
                        # diagonal kill: adds -3840 at col b*128+p
                        nc.tensor.matmul(
                            ps[:, nch_b * 512:(nch_b + 1) * 512],
                            zsel[:, :],
                            wdg[:, (NB - 1 - b) * 128 + nch_b * 512:
                                (NB - 1 - b) * 128 + (nch_b + 1) * 512],
                            start=False, stop=True)
                # EW = w_j * exp(S/tau); accum_out = AW row-sum (free on ACT).
                # ew must be F32: the ACT accumulator sums pre-downcast fp32
                # values, and den = AW' - BU' cancels catastrophically unless
                # the BU STT sums exactly the same values.
                ew = ew_pool.tile([128, TW], F32)
                nc.scalar.activation(ew[:, :], ps[:, :],
                                     mybir.ActivationFunctionType.Exp,
                                     scale=ESC,
                                     accum_out=awslt[:, (b * NS + t):
                                                     (b * NS + t) + 1])
                # BU': same-class row-sum of EW (diag already zero) -- DVE
                buscr = busc_pool.tile([128, TW], BF16, tag="buscr")
                nc.vector.scalar_tensor_tensor(
                    buscr[:, :], ybc[:, t * TW:(t + 1) * TW], yrow[:, b:b + 1],
                    ew[:, :],
                    op0=mybir.AluOpType.is_equal, op1=mybir.AluOpType.mult,
                    accum_out=buslt[:, (b * NS + t):(b * NS + t) + 1])

        # ---- finalize ----------------------------------------------------
        # fin layout: [den_in(NB) | num_in(NB) | den_l(NB) | num_l(NB)]
        fin = const.tile([128, 4 * NB], F32, tag="fin")
        awcol = const.tile([128, NB], F32, tag="awcol")
        bucol = const.tile([128, NB], F32, tag="bucol")
        for b in range(NB):
            nc.vector.tensor_reduce(awcol[:, b:b + 1], awslt[:, b * NS:(b + 1) * NS],
                                    mybir.AxisListType.X, mybir.AluOpType.add)
            nc.vector.tensor_reduce(bucol[:, b:b + 1], buslt[:, b * NS:(b + 1) * NS],
                                    mybir.AxisListType.X, mybir.AluOpType.add)
        # den_in = aw' - bu' ; num_in = kpos + c * bu'
        nc.vector.tensor_tensor(fin[:, 0:NB], awcol[:, :], bucol[:, :],
                                op=mybir.AluOpType.subtract)
        nc.vector.tensor_tensor(fin[:, NB:2 * NB], bucol[:, :], cloc[:, :],
                                op=mybir.AluOpType.mult)
        nc.vector.tensor_tensor(fin[:, NB:2 * NB], fin[:, NB:2 * NB],
                                kpos[:, :], op=mybir.AluOpType.add)
        # one Ln over both blocks
        nc.scalar.activation(fin[:, 2 * NB:4 * NB], fin[:, 0:2 * NB],
                             mybir.ActivationFunctionType.Ln)
        diff = const.tile([128, NB], F32, tag="diff")
        nc.vector.tensor_tensor(diff[:, :], fin[:, 2 * NB:3 * NB],
                                fin[:, 3 * NB:4 * NB], op=mybir.AluOpType.subtract)
        nc.vector.tensor_tensor(losscol[:, :], diff[:, :], dinv[:, :],
                                op=mybir.AluOpType.mult)

        # ---- reduce to a single partial ----------------------------------
        lsum = const.tile([128, 1], F32, tag="lsum")
        nc.vector.tensor_reduce(lsum[:, :], losscol[:, :],
                                mybir.AxisListType.X, mybir.AluOpType.add)
        psf = psum_pool.tile([128, TW], F32, tag="ps_t", name="psf")
        nc.tensor.matmul(psf[0:1, 0:1], lsum[:, :],
                         ones_col[:, :], start=True, stop=True)
        outsb = const.tile([1, 1], F32, tag="outsb")
        nc.scalar.copy(outsb[0:1, 0:1], psf[0:1, 0:1])
        nc.sync.dma_start(out_d[:, :], outsb[0:1, 0:1])

    nc.compile()
    return nc


# ---------------------------------------------------------------------------
# host-side marshalling
# ---------------------------------------------------------------------------

def make_inputs(q, k, y, cfg: Cfg):
    """Build the per-core input maps (pure layout/replication marshalling)."""
    N, D, KP, TW = cfg.N, cfg.D, cfg.KP, cfg.TW
    NL, NB, NS, KC = cfg.NL, cfg.NB, cfg.NS, cfg.KC
    q = np.asarray(q, dtype=np.float32)
    k = np.asarray(k, dtype=np.float32)
    y = np.asarray(y).astype(np.int64)
    KW = KP * 128
    FP8NP = ml_dtypes.float8_e4m3fn

    counts = np.bincount(y, minlength=NUM_CLASSES).astype(np.float64)
    w = 1.0 / np.maximum(counts, 1.0)                     # [C]
    # w-fold: drop q's last two feature dims and fold 256*tau*ln(w_j) into
    # the contraction (ones on the stationary side, X1+X2 on the moving side)
    X = (np.log(w[y]) * (QSCALE * QSCALE * TAU)).astype(np.float32)   # [N]
    X1 = X.astype(FP8NP)
    X2 = (X - X1.astype(np.float32)).astype(FP8NP)
    # moving side: q columns with dims 510/511 replaced by X1/X2
    q8m = (q * QSCALE).astype(FP8NP)                      # [N, D]
    q8m[:, D - 2] = X1
    q8m[:, D - 1] = X2
    # stationary side: q rows with dims 510/511 replaced by ones
    q8s = (q * QSCALE).astype(FP8NP)
    q8s[:, D - 2] = 1.0
    q8s[:, D - 1] = 1.0

    # wdg[p, t] = DIAG_W iff t == (NB-1)*128 + p (shifted identity window)
    WDGW = NL + (NB - 1) * 128
    wdg = np.zeros((128, WDGW), dtype=FP8NP)
    for qq in range(128):
        wdg[qq, (NB - 1) * 128 + qq] = DIAG_W
    zsel = np.zeros((128, 128), dtype=FP8NP)
    np.fill_diagonal(zsel, DIAG_Z)

    # mask8[p, m] = 1 iff m//KP == p (keep only own-row k entries)
    mask8 = np.zeros((128, KW), dtype=np.float16)
    for p in range(128):
        mask8[p, p * KP:(p + 1) * KP] = 1.0

    in_maps = []
    for r in range(cfg.ncores):
        rows = slice(r * NL, (r + 1) * NL)
        yl = y[rows]
        # rotated column permutation in NL units: unit u covers original
        # unit (r+u)%ncores, so the diagonal block heads column tile t=0
        NU = N // NL
        perm = np.concatenate(
            [np.arange(((r + u) % NU) * NL, ((r + u) % NU) * NL + NL)
             for u in range(NU)])
        qTr = np.ascontiguousarray(q8m[perm].T).reshape(KC, 128, N)
        qTl = np.ascontiguousarray(q8s[rows].T).reshape(KC, 128, NL)
        ybc = np.broadcast_to(y[perm].astype(np.float16)[None, :],
                              (128, N)).copy()
        # kT[b, c, dd, i*KP+kk] = k8[row b*128+i, kk, c*128+dd]
        # (dims 510/511 zeroed: the stationary ones-rows must not see k)
        k8 = (k[rows] * QSCALE).astype(FP8NP)
        k8[:, :, D - 2:D] = 0.0
        kl = k8.reshape(NB, 128, KP, KC, 128)
        kT = np.ascontiguousarray(
            kl.transpose(0, 3, 4, 1, 2).reshape(NB, KC, 128, KW))
        yrow = np.ascontiguousarray(yl.astype(np.float32).reshape(NB, 128).T)
        cl = counts[yl].reshape(NB, 128).T                # [128, NB]
        cloc = np.ascontiguousarray(cl).astype(np.float32)
        dinv = np.ascontiguousarray(1.0 / (cl - 1.0 + KP)).astype(np.float32)
        in_maps.append({
            "qTr": qTr, "qTl": qTl, "kT": kT,
            "ybc": ybc, "yrow": yrow, "wdg": wdg, "zsel": zsel,
            "mask8": mask8, "cloc": cloc, "dinv": dinv,
        })
    return in_maps


_CACHE = {}


def _get_nc(cfg_key):
    if cfg_key not in _CACHE:
        cfg = Cfg()
        _CACHE[cfg_key] = (cfg, build_bass(cfg))
    return _CACHE[cfg_key]


def kernel(q, k, y, trace=False):
    cfg, nc = _get_nc("full")
    in_maps = make_inputs(q, k, y, cfg)
    res = run_bass_kernel_spmd(nc, in_maps, core_ids=list(range(NCORES)),
                               trace=trace)
    total = np.sum([res.results[r]["out"][0, 0] for r in range(NCORES)],
                   dtype=np.float64)
    out = np.asarray(total / cfg.N, dtype=np.float32)
    if trace:
        kernel.last_results = res
    return out


# revision 35
# speedup vs baseline: 1.2180x; 1.2180x over previous
"""Trainium2 Bass kernel for the supervised-contrastive loss (nn_KCL_69784628626020).

Strategy (8 NeuronCores, SPMD, zero collectives):
  - Shard anchors (rows of q, k, y) across cores: 1024 rows/core.
  - Class counts / weights are computed on the HOST (pure input marshalling):
    no on-device histogram, no AllReduce, no cross-core sync at all.
  - Each core computes its [1024, 8192] slab of S = q_loc @ q_full^T with
    fp8e4 DoubleRow matmuls (q pre-scaled by 16; exp scale absorbs the 256x).
  - Column tiles are ROTATED per core (tile t covers original column tile
    (r+t) mod 8), so the diagonal block always lands in tile t=0 and the
    diagonal-kill matmul (-240*16 at the diagonal entry, pushing exp to 0)
    is issued for t=0 only.
  - The per-column weight w_j = 1/count(y_j) is folded INTO the contraction:
    q's last two feature dims are dropped (zero-mean noise comparable to the
    fp8 quantization) and replaced by ones on the stationary side and
    X1,X2 (fp8 value + residual of 256*tau*ln(w_j)) on the moving side, so
    S picks up +tau*ln(w_j) with ZERO extra matmuls. ScalarE then computes
    EW = w_j * exp(S/tau) directly and its accum_out produces
    AW_i = sum_j w_j E_ij for free.
  - Per row i and tile the only DVE pass is
        BU'_i += sum_{y_j==y_i} EW_ij    (STT is_equal+mult on DVE)
    with BU'_i = w_i * BU_i (w constant within a class).
  - den_i = log(AW_i - BU'_i); num_i = log(kpos_i + c_i*BU'_i)
    loss_i = (den_i - num_i) / (c_i - 1 + K)
  - kpos via fp8 DoubleRow matmuls + exp + masked STT row-reduce.
  - Final mean: ones-matmul partition reduction -> per-core partial; host
    adds the 8 partials (the unshard step).
"""

import numpy as np
from contextlib import ExitStack

import concourse.bass as bass
import concourse.bacc as bacc
import concourse.tile as tile
from concourse import mybir
from concourse.bass_utils import run_bass_kernel_spmd
import ml_dtypes

F32 = mybir.dt.float32
F16 = mybir.dt.float16
BF16 = mybir.dt.bfloat16
FP8 = mybir.dt.float8e4

TAU = 0.07
NCORES = 8
QSCALE = 16.0          # q,k pre-scale before fp8 quantization
DIAG_Z = -240.0        # zsel value (TRN fp8e4 max magnitude)
DIAG_W = 16.0          # wdg value; product -3840 kills the diagonal exp
NUM_CLASSES = 1000


class Cfg:
    def __init__(self, N=8192, D=512, KP=8, TW=2048, ncores=NCORES):
        self.N = N            # total rows (anchors)
        self.D = D            # feature dim
        self.KP = KP          # external positives per anchor
        self.TW = TW          # column tile width (2 PSUM banks wide x f32)
        self.ncores = ncores
        self.NL = N // ncores     # rows per core
        self.NB = self.NL // 128  # row blocks per core
        self.NS = N // TW         # column tiles
        self.KC = D // 128        # 128-contraction chunks
        assert self.NL % 128 == 0 and N % TW == 0 and D % 128 == 0
        assert TW % 512 == 0 and self.KC % 2 == 0
        assert TW % self.NL == 0  # diagonal lands in the head of tile t=0
        self.NCH = TW // 512      # 512-wide psum chunks per column tile
        assert KP * 128 <= 1024


def build_bass(cfg: Cfg, e_bufs=10):
    N, D, KP, TW = cfg.N, cfg.D, cfg.KP, cfg.TW
    NL, NB, NS, KC, NCH = cfg.NL, cfg.NB, cfg.NS, cfg.KC, cfg.NCH

    nc = bacc.Bacc("TRN2", target_bir_lowering=False, debug=False,
                   num_devices=cfg.ncores)

    KW = KP * 128  # k-path tile width per row block
    KWCH = KW // 512

    # ---- kernel I/O -------------------------------------------------------
    qTr_d = nc.dram_tensor("qTr", [KC, 128, N], FP8, kind="ExternalInput")
    qTl_d = nc.dram_tensor("qTl", [KC, 128, NL], FP8, kind="ExternalInput")
    kT_d = nc.dram_tensor("kT", [NB, KC, 128, KW], FP8, kind="ExternalInput")
    ybc_d = nc.dram_tensor("ybc", [128, N], F16, kind="ExternalInput")
    yrow_d = nc.dram_tensor("yrow", [128, NB], F32, kind="ExternalInput")
    wdg_d = nc.dram_tensor("wdg", [128, NL + (NB - 1) * 128], FP8,
                           kind="ExternalInput")
    zsel_d = nc.dram_tensor("zsel", [128, 128], FP8, kind="ExternalInput")
    mask8_d = nc.dram_tensor("mask8", [128, KW], F16, kind="ExternalInput")
    cloc_d = nc.dram_tensor("cloc", [128, NB], F32, kind="ExternalInput")
    dinv_d = nc.dram_tensor("dinv", [128, NB], F32, kind="ExternalInput")
    out_d = nc.dram_tensor("out", [1, 1], F32, kind="ExternalOutput")

    ESC = float(1.0 / (QSCALE * QSCALE * TAU))  # exp scale

    with tile.TileContext(nc) as tc, ExitStack() as ctx:
        const = ctx.enter_context(tc.tile_pool(name="const", bufs=1))
        rh_pool = ctx.enter_context(tc.tile_pool(name="rh", bufs=2))
        psum_pool = ctx.enter_context(tc.tile_pool(name="ps", bufs=2, space="PSUM"))
        ew_pool = ctx.enter_context(tc.tile_pool(name="ew", bufs=e_bufs))
        busc_pool = ctx.enter_context(tc.tile_pool(name="busc", bufs=2))
        kt_pool = ctx.enter_context(tc.tile_pool(name="kt", bufs=2))
        ek_pool = ctx.enter_context(tc.tile_pool(name="ek", bufs=2))

        # ---- resident constants (main-slab operands first so PE starts) --
        qtl = const.tile([128, KC, NL], FP8, tag="qtl")
        for c in range(KC):
            nc.sync.dma_start(qtl[:, c, :], qTl_d[c, :, :])
        zsel = const.tile([128, 128], FP8, tag="zsel")
        nc.sync.dma_start(zsel[:, :], zsel_d[:, :])
        wdg = const.tile([128, NL + (NB - 1) * 128], FP8, tag="wdg")
        nc.sync.dma_start(wdg[:, :], wdg_d[:, :])
        yrow = const.tile([128, NB], F32, tag="yrow")
        nc.sync.dma_start(yrow[:, :], yrow_d[:, :])
        ybc = const.tile([128, N], F16, tag="ybc")
        mask8 = const.tile([128, KW], F16, tag="mask8")
        cloc = const.tile([128, NB], F32, tag="cloc")
        dinv = const.tile([128, NB], F32, tag="dinv")
        ones_col = const.tile([128, 1], F32, tag="ones_col")
        nc.vector.memset(ones_col[:, :], 1.0)

        # accumulator slots
        awslt = const.tile([128, NB * NS], F32, tag="awslt")
        buslt = const.tile([128, NB * NS], F32, tag="buslt")
        kpos = const.tile([128, NB], F32, tag="kpos")
        losscol = const.tile([128, NB], F32, tag="losscol")

        # ---- unified loop: k-path blocks woven into the column-tile loop -
        KB_PER_T = NB // NS  # k-path row blocks processed per column tile

        def k_block(b):
            kt = kt_pool.tile([128, KC, KW], FP8, tag="kt", name=f"kt{b}")
            for c in range(KC):
                nc.sync.dma_start(kt[:, c, :], kT_d[b, c, :, :])
            kps = psum_pool.tile([128, TW], F32, name="kps", tag="ps_t")
            for dc in range(KC // 2):
                for nch in range(KWCH):
                    nc.tensor.matmul(
                        kps[:, nch * 512:(nch + 1) * 512],
                        qtl[:, 2 * dc:2 * dc + 2, b * 128:(b + 1) * 128],
                        kt[:, 2 * dc:2 * dc + 2, nch * 512:(nch + 1) * 512],
                        start=(dc == 0), stop=(dc == KC // 2 - 1),
                        perf_mode=mybir.MatmulPerfMode.DoubleRow)
            ek = ek_pool.tile([128, KW], BF16, tag="ek")
            nc.scalar.activation(ek[:, :], kps[:, 0:KW],
                                 mybir.ActivationFunctionType.Exp, scale=ESC)
            nc.vector.scalar_tensor_tensor(
                ek[:, :], mask8[:, :], 1.0, ek[:, :],
                op0=mybir.AluOpType.mult, op1=mybir.AluOpType.mult,
                accum_out=kpos[:, b:b + 1])

        for t in range(NS):
            rhs = rh_pool.tile([128, KC, TW], FP8, tag="rh", name=f"rhs{t}")
            for c in range(KC):
                nc.sync.dma_start(rhs[:, c, :], qTr_d[c, :, t * TW:(t + 1) * TW])
            # per-tile slice of the big/late constants
            nc.sync.dma_start(ybc[:, t * TW:(t + 1) * TW],
                              ybc_d[:, t * TW:(t + 1) * TW])
            if t == 0:
                nc.sync.dma_start(mask8[:, :], mask8_d[:, :])
                nc.sync.dma_start(cloc[:, :], cloc_d[:, :])
                nc.sync.dma_start(dinv[:, :], dinv_d[:, :])
            for b in range(NB):
                nch_b = (b * 128) // 512  # psum chunk holding the diagonal
                ps = psum_pool.tile([128, TW], F32, name="ps", tag="ps_t")
                for nch in range(NCH):
                    diag_here = (t == 0 and nch == nch_b)
                    for dc in range(KC // 2):
                        last = (dc == KC // 2 - 1)
                        nc.tensor.matmul(
                            ps[:, nch * 512:(nch + 1) * 512],
                            qtl[:, 2 * dc:2 * dc + 2, b * 128:(b + 1) * 128],
                            rhs[:, 2 * dc:2 * dc + 2, nch * 512:(nch + 1) * 512],
                            start=(dc == 0), stop=(last and not diag_here),
                            perf_mode=mybir.MatmulPerfMode.DoubleRow)
                    if diag_here:
                        # diagonal kill: adds -3840 at col b*128+p
                        nc.tensor.matmul(
                            ps[:, nch_b * 512:(nch_b + 1) * 512],
                            zsel[:, :],
                            wdg[:, (NB - 1 - b) * 128 + nch_b * 512:
                                (NB - 1 - b) * 128 + (nch_b + 1) * 512],
                            start=False, stop=True)
                # EW = w_j * exp(S/tau); accum_out = AW row-sum (free on ACT).
                # ew must be F32: the ACT accumulator sums pre-downcast fp32
                # values, and den = AW' - BU' cancels catastrophically unless
                # the BU STT sums exactly the same values.
                ew = ew_pool.tile([128, TW], F32)
                nc.scalar.activation(ew[:, :], ps[:, :],
                                     mybir.ActivationFunctionType.Exp,
                                     scale=ESC,
                                     accum_out=awslt[:, (b * NS + t):
                                                     (b * NS + t) + 1])
                # BU': same-class row-sum of EW (diag already zero) -- DVE
                buscr = busc_pool.tile([128, TW], BF16, tag="buscr")
                nc.vector.scalar_tensor_tensor(
                    buscr[:, :], ybc[:, t * TW:(t + 1) * TW], yrow[:, b:b + 1],
                    ew[:, :],
                    op0=mybir.AluOpType.is_equal, op1=mybir.AluOpType.mult,
                    accum_out=buslt[:, (b * NS + t):(b * NS + t) + 1])
            for kb in range(KB_PER_T * t, KB_PER_T * (t + 1)):
                k_block(kb)

        # ---- finalize ----------------------------------------------------
        # fin layout: [den_in(NB) | num_in(NB) | den_l(NB) | num_l(NB)]
        fin = const.tile([128, 4 * NB], F32, tag="fin")
        awcol = const.tile([128, NB], F32, tag="awcol")
        bucol = const.tile([128, NB], F32, tag="bucol")
        for b in range(NB):
            nc.vector.tensor_reduce(awcol[:, b:b + 1], awslt[:, b * NS:(b + 1) * NS],
                                    mybir.AxisListType.X, mybir.AluOpType.add)
            nc.vector.tensor_reduce(bucol[:, b:b + 1], buslt[:, b * NS:(b + 1) * NS],
                                    mybir.AxisListType.X, mybir.AluOpType.add)
        # den_in = aw' - bu' ; num_in = kpos + c * bu'
        nc.vector.tensor_tensor(fin[:, 0:NB], awcol[:, :], bucol[:, :],
                                op=mybir.AluOpType.subtract)
        nc.vector.tensor_tensor(fin[:, NB:2 * NB], bucol[:, :], cloc[:, :],
                                op=mybir.AluOpType.mult)
        nc.vector.tensor_tensor(fin[:, NB:2 * NB], fin[:, NB:2 * NB],
                                kpos[:, :], op=mybir.AluOpType.add)
        # one Ln over both blocks
        nc.scalar.activation(fin[:, 2 * NB:4 * NB], fin[:, 0:2 * NB],
                             mybir.ActivationFunctionType.Ln)
        diff = const.tile([128, NB], F32, tag="diff")
        nc.vector.tensor_tensor(diff[:, :], fin[:, 2 * NB:3 * NB],
                                fin[:, 3 * NB:4 * NB], op=mybir.AluOpType.subtract)
        nc.vector.tensor_tensor(losscol[:, :], diff[:, :], dinv[:, :],
                                op=mybir.AluOpType.mult)

        # ---- reduce to a single partial ----------------------------------
        lsum = const.tile([128, 1], F32, tag="lsum")
        nc.vector.tensor_reduce(lsum[:, :], losscol[:, :],
                                mybir.AxisListType.X, mybir.AluOpType.add)
        psf = psum_pool.tile([128, TW], F32, tag="ps_t", name="psf")
        nc.tensor.matmul(psf[0:1, 0:1], lsum[:, :],
                         ones_col[:, :], start=True, stop=True)
        outsb = const.tile([1, 1], F32, tag="outsb")
        nc.scalar.copy(outsb[0:1, 0:1], psf[0:1, 0:1])
        nc.sync.dma_start(out_d[:, :], outsb[0:1, 0:1])

    nc.compile()
    return nc


# ---------------------------------------------------------------------------
# host-side marshalling
# ---------------------------------------------------------------------------

def make_inputs(q, k, y, cfg: Cfg):
    """Build the per-core input maps (pure layout/replication marshalling)."""
    N, D, KP, TW = cfg.N, cfg.D, cfg.KP, cfg.TW
    NL, NB, NS, KC = cfg.NL, cfg.NB, cfg.NS, cfg.KC
    q = np.asarray(q, dtype=np.float32)
    k = np.asarray(k, dtype=np.float32)
    y = np.asarray(y).astype(np.int64)
    KW = KP * 128
    FP8NP = ml_dtypes.float8_e4m3fn

    counts = np.bincount(y, minlength=NUM_CLASSES).astype(np.float64)
    w = 1.0 / np.maximum(counts, 1.0)                     # [C]
    # w-fold: drop q's last two feature dims and fold 256*tau*ln(w_j) into
    # the contraction (ones on the stationary side, X1+X2 on the moving side)
    X = (np.log(w[y]) * (QSCALE * QSCALE * TAU)).astype(np.float32)   # [N]
    X1 = X.astype(FP8NP)
    X2 = (X - X1.astype(np.float32)).astype(FP8NP)
    # moving side: q columns with dims 510/511 replaced by X1/X2
    q8m = (q * QSCALE).astype(FP8NP)                      # [N, D]
    q8m[:, D - 2] = X1
    q8m[:, D - 1] = X2
    # stationary side: q rows with dims 510/511 replaced by ones
    q8s = (q * QSCALE).astype(FP8NP)
    q8s[:, D - 2] = 1.0
    q8s[:, D - 1] = 1.0

    # wdg[p, t] = DIAG_W iff t == (NB-1)*128 + p (shifted identity window)
    WDGW = NL + (NB - 1) * 128
    wdg = np.zeros((128, WDGW), dtype=FP8NP)
    for qq in range(128):
        wdg[qq, (NB - 1) * 128 + qq] = DIAG_W
    zsel = np.zeros((128, 128), dtype=FP8NP)
    np.fill_diagonal(zsel, DIAG_Z)

    # mask8[p, m] = 1 iff m//KP == p (keep only own-row k entries)
    mask8 = np.zeros((128, KW), dtype=np.float16)
    for p in range(128):
        mask8[p, p * KP:(p + 1) * KP] = 1.0

    in_maps = []
    for r in range(cfg.ncores):
        rows = slice(r * NL, (r + 1) * NL)
        yl = y[rows]
        # rotated column permutation in NL units: unit u covers original
        # unit (r+u)%ncores, so the diagonal block heads column tile t=0
        NU = N // NL
        perm = np.concatenate(
            [np.arange(((r + u) % NU) * NL, ((r + u) % NU) * NL + NL)
             for u in range(NU)])
        qTr = np.ascontiguousarray(q8m[perm].T).reshape(KC, 128, N)
        qTl = np.ascontiguousarray(q8s[rows].T).reshape(KC, 128, NL)
        ybc = np.broadcast_to(y[perm].astype(np.float16)[None, :],
                              (128, N)).copy()
        # kT[b, c, dd, i*KP+kk] = k8[row b*128+i, kk, c*128+dd]
        # (dims 510/511 zeroed: the stationary ones-rows must not see k)
        k8 = (k[rows] * QSCALE).astype(FP8NP)
        k8[:, :, D - 2:D] = 0.0
        kl = k8.reshape(NB, 128, KP, KC, 128)
        kT = np.ascontiguousarray(
            kl.transpose(0, 3, 4, 1, 2).reshape(NB, KC, 128, KW))
        yrow = np.ascontiguousarray(yl.astype(np.float32).reshape(NB, 128).T)
        cl = counts[yl].reshape(NB, 128).T                # [128, NB]
        cloc = np.ascontiguousarray(cl).astype(np.float32)
        dinv = np.ascontiguousarray(1.0 / (cl - 1.0 + KP)).astype(np.float32)
        in_maps.append({
            "qTr": qTr, "qTl": qTl, "kT": kT,
            "ybc": ybc, "yrow": yrow, "wdg": wdg, "zsel": zsel,
            "mask8": mask8, "cloc": cloc, "dinv": dinv,
        })
    return in_maps


_CACHE = {}


def _get_nc(cfg_key):
    if cfg_key not in _CACHE:
        cfg = Cfg()
        _CACHE[cfg_key] = (cfg, build_bass(cfg))
    return _CACHE[cfg_key]


def kernel(q, k, y, trace=False):
    cfg, nc = _get_nc("full")
    in_maps = make_inputs(q, k, y, cfg)
    res = run_bass_kernel_spmd(nc, in_maps, core_ids=list(range(NCORES)),
                               trace=trace)
    total = np.sum([res.results[r]["out"][0, 0] for r in range(NCORES)],
                   dtype=np.float64)
    out = np.asarray(total / cfg.N, dtype=np.float32)
    if trace:
        kernel.last_results = res
    return out
